# revision 1
# baseline (speedup 1.0000x reference)
"""DevignModel (GGNN message passing) Trainium2 kernel, 8 NeuronCores.

Strategy (graph/edge-cut parallelism per the sharding hint):
  - Nodes sharded contiguously across 8 cores (12800 padded rows each);
    h kept feature-major (h^T, fp16) resident in SBUF.
  - Per GGNN step: m = h @ W_t per 128-node tile on PE (fp16), written to
    a DRAM bounce buffer and AllGather'd per sub-shard (5 of them) so
    every core holds the full 102400-row message table.
  - Edges partitioned by dst core, bucketed by (2-tile group, src
    segment) so relative indices fit dma_gather's int16; dma_gather
    calls of <=8 chunks (SWDGE ring limit) on 4 queues.
  - Scatter-add computed transposed: aggT[feat,dst] += G_half^T @ S per
    chunk, accumulating both feature halves in one PSUM bank via a
    single start (lazy pending-zero, probe-validated). S[e,d] =
    (dst_rel[e]==d) built on DVE in one op per tile.
  - GRU in transposed [gate,node] layout, all matmuls fp16 (fp16's
    10-bit mantissa keeps the systematic weight-quantization error that
    bf16 suffers ~16x smaller); biases added as DVE broadcast ops;
    groups software-pipelined so scatter(g+1) hides GRU(g) latency.
  - Epilogue: h^T transposed back once, per-graph sums via one-hot
    matmul into PSUM; host sums the 8 partial [128,385] blocks and runs
    the tiny MLP.
"""

import numpy as np

import concourse.bass as bass
import concourse.bacc as bacc
import concourse.mybir as mybir
import concourse.tile as tile
from concourse import bass_utils, library_config

F32 = mybir.dt.float32
F16 = mybir.dt.float16
I16 = mybir.dt.int16

CORES = 8
P = 128
HID = 256
IN_DIM = 128

MSG_NP = np.float16


def _default_cfg():
    return dict(
        NREAL=100000,
        E=3200000,
        STEPS=6,
        NGRAPH=128,
        NLOC=12800,  # padded nodes per core (multiple of 128)
        NSEG=5,      # src segments = sub-shard AllGather pieces
        GT=2,        # dst tiles per gather group
        GCHUNK=8,    # max 128-row chunks per dma_gather call
    )


def _derived(cfg):
    c = dict(cfg)
    c["NLOCREAL"] = c["NREAL"] // CORES
    c["TILES"] = c["NLOC"] // P
    c["NPAD"] = c["NLOC"] * CORES
    c["SEGROWS"] = c["NPAD"] // c["NSEG"]
    c["SUBSH"] = c["NLOC"] // c["NSEG"]
    c["NGRP"] = c["TILES"] // c["GT"]
    assert c["SEGROWS"] <= 32768
    assert c["NLOC"] % P == 0 and c["NLOC"] % c["NSEG"] == 0
    assert c["SUBSH"] % P == 0 and c["TILES"] % c["NSEG"] == 0
    assert c["TILES"] % c["GT"] == 0
    return c


def preprocess(x, edge_index, batch, cfg):
    """Build per-core and shared device arrays. Returns (arrays, meta)."""
    c = _derived(cfg)
    NLR, NLOC, T, NSEG, GT = (
        c["NLOCREAL"], c["NLOC"], c["TILES"], c["NSEG"], c["GT"])
    SUBSH = c["SUBSH"]

    x = np.asarray(x, dtype=np.float32)
    src = np.asarray(edge_index[0], dtype=np.int64)
    dst = np.asarray(edge_index[1], dtype=np.int64)
    batch = np.asarray(batch, dtype=np.int64)

    # real node id -> padded id
    s_pad = (src // NLR) * NLOC + src % NLR
    d_pad = (dst // NLR) * NLOC + dst % NLR

    ecore = d_pad // NLOC
    iloc = (d_pad % NLOC) // P          # dst tile within core
    drel = (d_pad % P).astype(np.float32)
    s_rank = s_pad // NLOC
    s_loc = s_pad % NLOC
    seg = (s_loc // SUBSH).astype(np.int64)
    srel = (s_rank * SUBSH + s_loc % SUBSH).astype(np.int64)

    # idx bucket order: (group, seg, tile-in-group) so one gather call's
    # buckets are contiguous
    grp = iloc // GT
    ti = iloc % GT
    b_core = (grp * NSEG + seg) * GT + ti
    nb = T * NSEG
    bucket = ecore * nb + b_core
    order = np.argsort(bucket, kind="stable")
    counts = np.bincount(bucket, minlength=CORES * nb)
    starts = np.zeros(CORES * nb + 1, dtype=np.int64)
    np.cumsum(counts, out=starts[1:])
    pos = np.arange(len(src)) - starts[bucket[order]]

    # per-bucket chunk count = max over cores, >= 1
    cc = counts.reshape(CORES, nb)
    ch_b = np.maximum(1, (cc.max(axis=0) + P - 1) // P)  # [T*NSEG]
    cap_b = ch_b * P
    cap_off = np.zeros(nb + 1, dtype=np.int64)
    np.cumsum(cap_b, out=cap_off[1:])
    CAPSUM = int(cap_off[-1])

    idx_cap = np.zeros((CORES, CAPSUM), dtype=np.int16)
    drel_cap = np.full((CORES, CAPSUM), 200.0, dtype=np.float32)
    bs = bucket[order]
    bc = bs // nb
    bl = bs % nb
    flatpos = cap_off[bl] + pos
    idx_cap[bc, flatpos] = srel[order].astype(np.int16)
    drel_cap[bc, flatpos] = drel[order]

    # pack idx: bucket b occupies cols [cap_off[b]//16, cap_off[b+1]//16)
    idx_packed = np.zeros((CORES, P, CAPSUM // 16), dtype=np.int16)
    v16 = idx_cap.reshape(CORES, CAPSUM // 16, 16)
    idx_packed[:] = np.tile(v16.transpose(0, 2, 1), (1, 8, 1))

    # dstrel packed in PER-TILE order (tile-major, segs concatenated) so
    # the S-matrix for a whole tile builds in one DVE op; [p, j] =
    # drel of that tile's j-th chunk, edge j*128+p.
    chunk_src = []  # bucket index per chunk, in dr order
    for i in range(T):
        g_, t_ = i // GT, i % GT
        for s in range(NSEG):
            b = (g_ * NSEG + s) * GT + t_
            chunk_src.extend([b] * int(ch_b[b]))
    chunk_src = np.asarray(chunk_src, dtype=np.int64)
    CHSUM = int(ch_b.sum())
    assert len(chunk_src) == CHSUM
    nth = np.zeros(nb, dtype=np.int64)
    firstrow = np.zeros(CHSUM, dtype=np.int64)
    for j in range(CHSUM):
        b = chunk_src[j]
        firstrow[j] = cap_off[b] + nth[b] * P
        nth[b] += 1
    rows = firstrow[:, None] + np.arange(P)[None, :]     # [CHSUM, 128]
    dr_packed = drel_cap[:, rows]                        # [C, CHSUM, 128]
    dr_packed = np.ascontiguousarray(
        dr_packed.transpose(0, 2, 1)).astype(MSG_NP)     # [C, 128, CHSUM]

    # x per core, padded; fp16 with trailing ones column for counts
    x_loc = np.zeros((CORES, NLOC, IN_DIM), dtype=np.float32)
    x_loc[:, :NLR] = x.reshape(CORES, NLR, IN_DIM)
    xr = x_loc.reshape(CORES, T, P, IN_DIM)
    xaug = np.ones((CORES, T, P, IN_DIM + 1), dtype=np.float32)
    xaug[..., :IN_DIM] = xr
    xaug = xaug.astype(MSG_NP)

    # h0^T resident layout: [128, T*256]; cols t*256 + k*128 + n
    h0t = np.zeros((CORES, P, T, 2, P), dtype=np.float32)
    h0t[:, :, :, 0, :] = xr.transpose(0, 3, 1, 2)
    h0t = h0t.reshape(CORES, P, T * HID).astype(MSG_NP)

    # batch rel (f32 for tensor_scalar is_equal), pads get 200
    b_loc = np.full((CORES, NLOC), 200.0, dtype=np.float32)
    b_loc[:, :NLR] = batch.reshape(CORES, NLR).astype(np.float32)
    batchrel = np.ascontiguousarray(
        b_loc.reshape(CORES, T, P).transpose(0, 2, 1))   # [C, 128, T]

    arrays = dict(
        idx=idx_packed, dstrel=dr_packed, xaug=xaug, h0t=h0t,
        batchrel=batchrel,
    )
    meta = dict(cfg=c, ch_b=ch_b, cap_off=cap_off, CAPSUM=CAPSUM,
                CHSUM=CHSUM)
    return arrays, meta


def shared_arrays(ggnn_weight, w_ih, w_hh, b_ih, b_hh, steps):
    f16 = MSG_NP
    gg = np.asarray(ggnn_weight, np.float32).reshape(steps, 2, P, HID)
    w_all = np.ascontiguousarray(
        gg.transpose(2, 0, 1, 3).reshape(P, steps * 2 * HID))
    # wihT[f, (k*6+c)*128+g] = w_ih[c*128+g, k*128+f]
    wih = np.asarray(w_ih, np.float32).reshape(6, P, 2, P)
    wihT = np.ascontiguousarray(wih.transpose(3, 2, 0, 1).reshape(P, 12 * P))
    whh = np.asarray(w_hh, np.float32).reshape(6, P, 2, P)
    whhT = np.ascontiguousarray(whh.transpose(3, 2, 0, 1).reshape(P, 12 * P))
    b_ih = np.asarray(b_ih, np.float32)
    b_hh = np.asarray(b_hh, np.float32)
    brz = (b_ih + b_hh)[:512].reshape(4, P).T          # [128, 4]
    bin_ = b_ih[512:].reshape(2, P).T                  # [128, 2]
    bhn = b_hh[512:].reshape(2, P).T                   # [128, 2]
    iota = np.tile(np.arange(P, dtype=np.float32), (P, 1))
    return dict(
        wihT=wihT.astype(f16), whhT=whhT.astype(f16),
        w_all=w_all.astype(f16),
        b_rz=np.ascontiguousarray(brz), b_in=np.ascontiguousarray(bin_),
        b_hn=np.ascontiguousarray(bhn),
        iota_f16=iota.astype(f16),
        iota_f32=iota,
        ident_f16=np.eye(P, dtype=np.float32).astype(f16),
    )


def build(meta):
    c = meta["cfg"]
    T, NSEG, SEGROWS, NLOC, STEPS, GT, NGRP = (
        c["TILES"], c["NSEG"], c["SEGROWS"], c["NLOC"], c["STEPS"],
        c["GT"], c["NGRP"])
    GCH = c["GCHUNK"]
    SUBSH = c["SUBSH"]
    TPS = T // NSEG  # tiles per sub-shard
    ch_b = meta["ch_b"]
    cap_off = meta["cap_off"]
    CAPSUM, CHSUM = meta["CAPSUM"], meta["CHSUM"]

    def bidx(g, s, t_):
        return (g * NSEG + s) * GT + t_

    # per-tile chunk count and per-tile dr col offsets (dr is tile-major)
    chT = np.array([int(sum(ch_b[bidx(i // GT, s, i % GT)]
                            for s in range(NSEG))) for i in range(T)])
    chT_off = np.zeros(T + 1, dtype=np.int64)
    np.cumsum(chT, out=chT_off[1:])
    CTMAX = int(chT.max())
    GMAX = max(int(sum(ch_b[bidx(g, s, t_)] for t_ in range(GT)))
               for g in range(NGRP) for s in range(NSEG))

    nc = bacc.Bacc("TRN2", target_bir_lowering=False, debug=False,
                   num_devices=CORES, num_swdge_queues=4)

    idx_d = nc.dram_tensor("idx", [P, CAPSUM // 16], I16, kind="ExternalInput")
    dr_d = nc.dram_tensor("dstrel", [P, CHSUM], F16, kind="ExternalInput")
    xaug_d = nc.dram_tensor("xaug", [T, P, IN_DIM + 1], F16,
                            kind="ExternalInput")
    h0t_d = nc.dram_tensor("h0t", [P, T * HID], F16, kind="ExternalInput")
    batch_d = nc.dram_tensor("batchrel", [P, T], F32, kind="ExternalInput")
    wall_d = nc.dram_tensor("w_all", [P, STEPS * 2 * HID], F16,
                            kind="ExternalInput")
    wih_d = nc.dram_tensor("wihT", [P, 12 * P], F16, kind="ExternalInput")
    whh_d = nc.dram_tensor("whhT", [P, 12 * P], F16, kind="ExternalInput")
    brz_d = nc.dram_tensor("b_rz", [P, 4], F32, kind="ExternalInput")
    bin_d = nc.dram_tensor("b_in", [P, 2], F32, kind="ExternalInput")
    bhn_d = nc.dram_tensor("b_hn", [P, 2], F32, kind="ExternalInput")
    iotah_d = nc.dram_tensor("iota_f16", [P, P], F16, kind="ExternalInput")
    iotaf_d = nc.dram_tensor("iota_f32", [P, P], F32, kind="ExternalInput")
    identh_d = nc.dram_tensor("ident_f16", [P, P], F16, kind="ExternalInput")
    pool_d = nc.dram_tensor("pool", [P, IN_DIM + 1 + HID], F32,
                            kind="ExternalOutput")

    with tile.TileContext(nc) as tc:
        with (
            tc.tile_pool(name="const", bufs=1) as cpool,
            tc.tile_pool(name="stream", bufs=4) as stpool,
            tc.tile_pool(name="gq", bufs=3) as gpool,
            tc.tile_pool(name="Sp", bufs=3) as spool,
            tc.tile_pool(name="work", bufs=3) as wpool,
            tc.tile_pool(name="dram", bufs=1, space="DRAM") as dpool,
        ):
            nc.gpsimd.load_library(library_config.mlp)

            def load_const(name, dram, shape, dtype):
                t_ = cpool.tile(shape, dtype, name=name)
                nc.sync.dma_start(out=t_[:], in_=dram)
                return t_

            wih_sb = load_const("wih", wih_d[:, :], [P, 12 * P], F16)
            whh_sb = load_const("whh", whh_d[:, :], [P, 12 * P], F16)
            wall_sb = load_const("wall", wall_d[:, :], [P, STEPS * 2 * HID],
                                 F16)
            brz_sb = load_const("brz", brz_d[:, :], [P, 4], F32)
            bin_sb = load_const("bin", bin_d[:, :], [P, 2], F32)
            bhn_sb = load_const("bhn", bhn_d[:, :], [P, 2], F32)
            iotah_sb = load_const("iotah", iotah_d[:, :], [P, P], F16)
            iotaf_sb = load_const("iotaf", iotaf_d[:, :], [P, P], F32)
            identh_sb = load_const("identh", identh_d[:, :], [P, P], F16)
            batch_sb = load_const("batch", batch_d[:, :], [P, T], F32)

            hT = []
            for i in range(T):
                t_ = cpool.tile([P, HID], F16, name=f"hT{i}")
                nc.sync.dma_start(out=t_[:], in_=h0t_d[:, i * HID:(i + 1) * HID])
                hT.append(t_)

            swdge_ctr = [0]

            cc_ins = [[dpool.tile([SUBSH, HID], F16, name=f"cc_in{t}_{k}")
                       for k in range(NSEG)] for t in range(STEPS)]
            cc_outs = [[dpool.tile([SEGROWS, HID], F16,
                                   name=f"cc_out{t}_{k}", addr_space="Shared")
                        for k in range(NSEG)] for t in range(STEPS)]

            with (
                tc.tile_pool(name="accps", bufs=4, space="PSUM") as accps,
                tc.tile_pool(name="rzps", bufs=2, space="PSUM") as rzps,
                tc.tile_pool(name="nps", bufs=2, space="PSUM") as nps,
            ):
                def gru_tile(t, i, agg_ps):
                    """GRU for dst tile i at step t; aggT in PSUM
                    [feat, (k, node)] layout."""
                    aggT_sb = wpool.tile([P, HID], F16, tag="aggT",
                                         name=f"aT{t}_{i}")
                    nc.scalar.copy(out=aggT_sb[:], in_=agg_ps[:])

                    rz_ps = rzps.tile([P, 512], F32, tag="rz",
                                      name=f"rz{t}_{i}")
                    for cch in range(4):
                        o = rz_ps[:, cch * P:(cch + 1) * P]
                        for k in range(2):
                            nc.tensor.matmul(
                                out=o,
                                lhsT=wih_sb[:, (k * 6 + cch) * P:
                                            (k * 6 + cch + 1) * P],
                                rhs=aggT_sb[:, k * P:(k + 1) * P],
                                start=(cch == 0 and k == 0), stop=False,
                                skip_group_check=True)
                        for k in range(2):
                            nc.tensor.matmul(
                                out=o,
                                lhsT=whh_sb[:, (k * 6 + cch) * P:
                                            (k * 6 + cch + 1) * P],
                                rhs=hT[i][:, k * P:(k + 1) * P],
                                start=False, stop=(cch == 3 and k == 1),
                                skip_group_check=True)
                    # rz + bias (broadcast over nodes), then sigmoid
                    rzb = wpool.tile([P, 512], F16, tag="rzb",
                                     name=f"rzb{t}_{i}")
                    nc.vector.tensor_tensor(
                        out=rzb[:].rearrange("p (c n) -> p c n", n=P),
                        in0=rz_ps[:].rearrange("p (c n) -> p c n", n=P),
                        in1=brz_sb[:].rearrange("p (c o) -> p c o", o=1)
                        .to_broadcast([P, 4, P]),
                        op=mybir.AluOpType.add)
                    rz_sb = wpool.tile([P, 512], F16, tag="rzsb",
                                       name=f"rzs{t}_{i}")
                    nc.scalar.activation(
                        out=rz_sb[:], in_=rzb[:],
                        func=mybir.ActivationFunctionType.Sigmoid)

                    inhn_ps = nps.tile([P, 512], F32, tag="n",
                                       name=f"n{t}_{i}")
                    for cch in range(2):
                        o = inhn_ps[:, cch * P:(cch + 1) * P]
                        for k in range(2):
                            nc.tensor.matmul(
                                out=o,
                                lhsT=wih_sb[:, (k * 6 + 4 + cch) * P:
                                            (k * 6 + 4 + cch + 1) * P],
                                rhs=aggT_sb[:, k * P:(k + 1) * P],
                                start=(cch == 0 and k == 0), stop=False,
                                skip_group_check=True)
                    for cch in range(2):
                        o = inhn_ps[:, 256 + cch * P:256 + (cch + 1) * P]
                        for k in range(2):
                            nc.tensor.matmul(
                                out=o,
                                lhsT=whh_sb[:, (k * 6 + 4 + cch) * P:
                                            (k * 6 + 4 + cch + 1) * P],
                                rhs=hT[i][:, k * P:(k + 1) * P],
                                start=False,
                                stop=(cch == 1 and k == 1),
                                skip_group_check=True)
                    # hnb = h_n + b_hn ; inb = i_n + b_in
                    hnb = wpool.tile([P, HID], F16, tag="tmp",
                                     name=f"hb{t}_{i}", bufs=6)
                    nc.vector.tensor_tensor(
                        out=hnb[:].rearrange("p (c n) -> p c n", n=P),
                        in0=inhn_ps[:, 256:512]
                        .rearrange("p (c n) -> p c n", n=P),
                        in1=bhn_sb[:].rearrange("p (c o) -> p c o", o=1)
                        .to_broadcast([P, 2, P]),
                        op=mybir.AluOpType.add)
                    inb = wpool.tile([P, HID], F16, tag="tmp",
                                     name=f"ib{t}_{i}", bufs=6)
                    nc.vector.tensor_tensor(
                        out=inb[:].rearrange("p (c n) -> p c n", n=P),
                        in0=inhn_ps[:, 0:256]
                        .rearrange("p (c n) -> p c n", n=P),
                        in1=bin_sb[:].rearrange("p (c o) -> p c o", o=1)
                        .to_broadcast([P, 2, P]),
                        op=mybir.AluOpType.add)
                    t1 = wpool.tile([P, HID], F16, tag="tmp",
                                    name=f"t1{t}_{i}", bufs=6)
                    nc.vector.tensor_mul(out=t1[:], in0=rz_sb[:, 0:HID],
                                         in1=hnb[:])
                    t2 = wpool.tile([P, HID], F16, tag="tmp",
                                    name=f"t2{t}_{i}", bufs=6)
                    nc.vector.tensor_add(out=t2[:], in0=t1[:], in1=inb[:])
                    n_sb = wpool.tile([P, HID], F16, tag="tmp",
                                      name=f"ns{t}_{i}", bufs=6)
                    nc.scalar.activation(
                        out=n_sb[:], in_=t2[:],
                        func=mybir.ActivationFunctionType.Tanh)
                    d_sb = wpool.tile([P, HID], F16, tag="tmp",
                                      name=f"d{t}_{i}", bufs=6)
                    nc.vector.tensor_sub(out=d_sb[:], in0=hT[i][:],
                                         in1=n_sb[:])
                    zd = wpool.tile([P, HID], F16, tag="tmp",
                                    name=f"zd{t}_{i}", bufs=6)
                    nc.vector.tensor_mul(out=zd[:], in0=rz_sb[:, HID:512],
                                         in1=d_sb[:])
                    nc.vector.tensor_add(out=hT[i][:], in0=zd[:], in1=n_sb[:])

                def scatter_group(t, g):
                    """Gathers + S-build + transposed scatter for group g.
                    Returns the GT aggT PSUM tiles."""
                    i0 = g * GT
                    dr0 = int(chT_off[i0])
                    dr1 = int(chT_off[i0 + GT])
                    dr_sb = stpool.tile([P, GT * CTMAX], F16, tag="dr",
                                        name=f"dr{t}_{g}")
                    nc.sync.dma_start(out=dr_sb[:, 0:dr1 - dr0],
                                      in_=dr_d[:, dr0:dr1])

                    Gs = []
                    for s in range(NSEG):
                        b0 = bidx(g, s, 0)
                        ic0 = int(cap_off[b0] // 16)
                        ic1 = int(cap_off[b0 + GT] // 16)
                        nch = (int(cap_off[b0 + GT]) - int(cap_off[b0])) // P
                        idx_sb = stpool.tile([P, GMAX * 8], I16, tag="idx",
                                             name=f"ix{t}_{g}_{s}")
                        nc.sync.dma_start(out=idx_sb[:, 0:ic1 - ic0],
                                          in_=idx_d[:, ic0:ic1])
                        G = gpool.tile([P, GMAX * HID], F16, tag="g",
                                       name=f"g{t}_{g}_{s}")
                        c0 = 0
                        while c0 < nch:
                            nsub = min(GCH, nch - c0)
                            nc.gpsimd.dma_gather(
                                G[:, c0 * HID:(c0 + nsub) * HID]
                                .rearrange("p (j d) -> p j d", d=HID),
                                cc_outs[t][s][:, :],
                                idx_sb[:, c0 * 8:(c0 + nsub) * 8],
                                nsub * P, nsub * P, HID,
                                queue_num=swdge_ctr[0] % 4)
                            swdge_ctr[0] += 1
                            c0 += nsub
                        Gs.append(G)

                    # one S build per tile (dr is tile-major)
                    S_tiles = []
                    for t_ in range(GT):
                        i = i0 + t_
                        cs = int(chT[i])
                        S_sb = spool.tile([P, CTMAX * P], F16, tag="S",
                                          name=f"S{t}_{g}_{t_}")
                        bdr = int(chT_off[i]) - dr0
                        nc.vector.tensor_tensor(
                            out=S_sb[:, :cs * P]
                            .rearrange("p (j d) -> p j d", d=P),
                            in0=dr_sb[:, bdr:bdr + cs]
                            .to_broadcast([P, cs, P]),
                            in1=iotah_sb[:]
                            .rearrange("p (a b) -> p a b", a=1)
                            .to_broadcast([P, cs, P]),
                            op=mybir.AluOpType.is_equal)
                        S_tiles.append(S_sb)

                    aggs = []
                    for t_ in range(GT):
                        i = i0 + t_
                        agg = accps.tile([P, HID], F32, tag="acc",
                                         name=f"ag{t}_{g}_{t_}")
                        aggs.append(agg)
                        nchunks = int(chT[i])
                        done = 0
                        scol = 0
                        for s in range(NSEG):
                            b = bidx(g, s, t_)
                            cs = int(ch_b[b])
                            joff = (int(cap_off[b]) -
                                    int(cap_off[bidx(g, s, 0)])) // P
                            for j in range(cs):
                                gcol = (joff + j) * HID
                                for k in range(2):
                                    nc.tensor.matmul(
                                        out=agg[:, k * P:(k + 1) * P],
                                        lhsT=Gs[s][:, gcol + k * P:
                                                   gcol + (k + 1) * P],
                                        rhs=S_tiles[t_][:, (scol + j) * P:
                                                        (scol + j + 1) * P],
                                        start=(done == 0 and k == 0),
                                        stop=(done == nchunks - 1 and k == 1),
                                        skip_group_check=True)
                                done += 1
                            scol += cs
                    return aggs

                for t in range(STEPS):
                    # ---- phase M: messages + AllGather ----
                    woff = t * 2 * HID
                    for i in range(T):
                        m_ps = accps.tile([P, HID], F32, tag="acc",
                                          name=f"mp{t}_{i}")
                        nc.tensor.matmul(
                            out=m_ps[:], lhsT=hT[i][:, 0:P],
                            rhs=wall_sb[:, woff:woff + HID],
                            start=True, stop=False)
                        nc.tensor.matmul(
                            out=m_ps[:], lhsT=hT[i][:, P:HID],
                            rhs=wall_sb[:, woff + HID:woff + 2 * HID],
                            start=False, stop=True)
                        m_sb = wpool.tile([P, HID], F16, tag="m",
                                          name=f"m{t}_{i}")
                        nc.scalar.copy(out=m_sb[:], in_=m_ps[:])
                        k = i // TPS
                        ioff = (i % TPS) * P
                        nc.sync.dma_start(
                            out=cc_ins[t][k][ioff:ioff + P, :], in_=m_sb[:])
                        if i % TPS == TPS - 1:
                            nc.gpsimd.collective_compute(
                                "AllGather", mybir.AluOpType.bypass,
                                replica_groups=[list(range(CORES))],
                                ins=[cc_ins[t][k].opt()],
                                outs=[cc_outs[t][k].opt()])

                    # ---- phase A: software-pipelined groups ----
                    prev = None
                    for g in range(NGRP):
                        aggs = scatter_group(t, g)
                        if prev is not None:
                            pg, paggs = prev
                            for t_ in range(GT):
                                gru_tile(t, pg * GT + t_, paggs[t_])
                        prev = (g, aggs)
                    pg, paggs = prev
                    for t_ in range(GT):
                        gru_tile(t, pg * GT + t_, paggs[t_])

            # ---- phase C: per-graph pooling partials ----
            with (
                tc.tile_pool(name="poolps", bufs=1, space="PSUM") as ppsC,
                tc.tile_pool(name="hfps", bufs=2, space="PSUM") as hfps,
            ):
                px_ps = ppsC.tile([P, IN_DIM + 1], F32, name="px")
                ph_ps = ppsC.tile([P, HID], F32, name="ph")
                for i in range(T):
                    hf_ps = hfps.tile([P, HID], F16, tag="hf", name=f"hf{i}")
                    nc.tensor.transpose(out=hf_ps[:, 0:P],
                                        in_=hT[i][:, 0:P],
                                        identity=identh_sb[:])
                    nc.tensor.transpose(out=hf_ps[:, P:HID],
                                        in_=hT[i][:, P:HID],
                                        identity=identh_sb[:])
                    hfin_sb = wpool.tile([P, HID], F16, tag="hfin",
                                         name=f"hfs{i}")
                    nc.scalar.copy(out=hfin_sb[:], in_=hf_ps[:])
                    sg_sb = wpool.tile([P, P], F16, tag="sg", name=f"sg{i}")
                    nc.vector.tensor_scalar(
                        out=sg_sb[:], in0=iotaf_sb[:],
                        scalar1=batch_sb[:, i:i + 1], scalar2=None,
                        op0=mybir.AluOpType.is_equal)
                    xo_sb = stpool.tile([P, IN_DIM + 1], F16, tag="xo",
                                        name=f"xo{i}")
                    nc.sync.dma_start(out=xo_sb[:], in_=xaug_d[i, :, :])
                    nc.tensor.matmul(out=px_ps[:], lhsT=sg_sb[:], rhs=xo_sb[:],
                                     start=(i == 0), stop=(i == T - 1))
                    nc.tensor.matmul(out=ph_ps[:], lhsT=sg_sb[:],
                                     rhs=hfin_sb[:],
                                     start=(i == 0), stop=(i == T - 1))
                px_sb = wpool.tile([P, IN_DIM + 1], F32, name="pxs")
                nc.vector.tensor_copy(out=px_sb[:], in_=px_ps[:])
                ph_sb = wpool.tile([P, HID], F32, name="phs")
                nc.vector.tensor_copy(out=ph_sb[:], in_=ph_ps[:])
                nc.sync.dma_start(out=pool_d[:, 0:IN_DIM + 1], in_=px_sb[:])
                nc.sync.dma_start(out=pool_d[:, IN_DIM + 1:], in_=ph_sb[:])

    nc.compile()
    return nc


_CACHE = {}


def run_device(inputs, cfg, trace=False, tmpdir=None):
    """Preprocess, compile (cached), run on 8 cores; returns per-core pool
    partials [CORES, 128, 385] plus the BassKernelResults."""
    c = _derived(cfg)
    arrays, meta = preprocess(
        inputs["x"], inputs["edge_index"], inputs["batch"], cfg)
    shared = shared_arrays(
        inputs["ggnn_weight"], inputs["w_ih"], inputs["w_hh"],
        inputs["b_ih"], inputs["b_hh"], c["STEPS"])

    key = (tuple(meta["ch_b"].tolist()), c["STEPS"], c["NLOC"], c["GT"],
           c["GCHUNK"])
    if key not in _CACHE:
        _CACHE[key] = build(meta)
    nc = _CACHE[key]

    in_maps = []
    for core in range(CORES):
        m = {k: np.ascontiguousarray(v[core]) for k, v in arrays.items()}
        m.update(shared)
        in_maps.append(m)
    kw = {}
    if trace:
        kw = dict(trace=True, tmpdir=tmpdir)
    res = bass_utils.run_bass_kernel_spmd(
        nc, in_maps, core_ids=list(range(CORES)), **kw)
    pool = np.stack([res.results[cr]["pool"] for cr in range(CORES)])
    return pool, res


def host_epilogue(pool, inputs):
    """Sum per-core partials, mean-pool, and run the classifier MLP."""
    tot = pool.sum(axis=0, dtype=np.float64).astype(np.float32)
    xsum = tot[:, :IN_DIM]
    cnt = tot[:, IN_DIM]
    hsum = tot[:, IN_DIM + 1:]
    feat = np.concatenate([xsum, hsum], axis=1)
    pooled = feat / np.maximum(cnt, 1.0)[:, None]
    w1 = np.asarray(inputs["mlp_w1"], np.float32)
    b1 = np.asarray(inputs["mlp_b1"], np.float32)
    w2 = np.asarray(inputs["mlp_w2"], np.float32)
    b2 = np.asarray(inputs["mlp_b2"], np.float32)
    hdn = np.maximum(pooled @ w1.T + b1, 0.0)
    return (hdn @ w2.T + b2).astype(np.float32)


def kernel(**inputs):
    cfg = _default_cfg()
    pool, _ = run_device(inputs, cfg)
    return host_epilogue(pool, inputs)



# revision 3
# speedup vs baseline: 1.0074x; 1.0074x over previous
"""DevignModel (GGNN message passing) Trainium2 kernel, 8 NeuronCores.

Strategy (graph/edge-cut parallelism per the sharding hint):
  - Nodes sharded contiguously across 8 cores (12800 padded rows each);
    h kept feature-major (h^T, fp16) resident in SBUF.
  - Per GGNN step: m = h @ W_t per 128-node tile on PE (fp16), written to
    a DRAM bounce buffer and AllGather'd per sub-shard (5 of them) so
    every core holds the full 102400-row message table.
  - Edges partitioned by dst core, bucketed by (2-tile group, src
    segment) so relative indices fit dma_gather's int16; dma_gather
    calls of <=8 chunks (SWDGE ring limit) on 4 queues.
  - Scatter-add computed transposed: aggT[feat,dst] += G_half^T @ S per
    chunk, accumulating both feature halves in one PSUM bank via a
    single start (lazy pending-zero, probe-validated). S[e,d] =
    (dst_rel[e]==d) built on DVE in one op per tile.
  - GRU in transposed [gate,node] layout, all matmuls fp16 (fp16's
    10-bit mantissa keeps the systematic weight-quantization error that
    bf16 suffers ~16x smaller); biases added as DVE broadcast ops;
    groups software-pipelined so scatter(g+1) hides GRU(g) latency.
  - Epilogue: h^T transposed back once, per-graph sums via one-hot
    matmul into PSUM; host sums the 8 partial [128,385] blocks and runs
    the tiny MLP.
"""

import numpy as np

import concourse.bass as bass
import concourse.bacc as bacc
import concourse.mybir as mybir
import concourse.tile as tile
from concourse import bass_utils, library_config

F32 = mybir.dt.float32
F16 = mybir.dt.float16
I16 = mybir.dt.int16

CORES = 8
P = 128
HID = 256
IN_DIM = 128

MSG_NP = np.float16


def _default_cfg():
    return dict(
        NREAL=100000,
        E=3200000,
        STEPS=6,
        NGRAPH=128,
        NLOC=12800,  # padded nodes per core (multiple of 128)
        NSEG=5,      # src segments = sub-shard AllGather pieces
        GT=2,        # dst tiles per gather group
        GCHUNK=8,    # max 128-row chunks per dma_gather call
    )


def _derived(cfg):
    c = dict(cfg)
    c["NLOCREAL"] = c["NREAL"] // CORES
    c["TILES"] = c["NLOC"] // P
    c["NPAD"] = c["NLOC"] * CORES
    c["SEGROWS"] = c["NPAD"] // c["NSEG"]
    c["SUBSH"] = c["NLOC"] // c["NSEG"]
    c["NGRP"] = c["TILES"] // c["GT"]
    assert c["SEGROWS"] <= 32768
    assert c["NLOC"] % P == 0 and c["NLOC"] % c["NSEG"] == 0
    assert c["SUBSH"] % P == 0 and c["TILES"] % c["NSEG"] == 0
    assert c["TILES"] % c["GT"] == 0
    return c


def preprocess(x, edge_index, batch, cfg):
    """Build per-core and shared device arrays. Returns (arrays, meta)."""
    c = _derived(cfg)
    NLR, NLOC, T, NSEG, GT = (
        c["NLOCREAL"], c["NLOC"], c["TILES"], c["NSEG"], c["GT"])
    SUBSH = c["SUBSH"]

    x = np.asarray(x, dtype=np.float32)
    src = np.asarray(edge_index[0], dtype=np.int64)
    dst = np.asarray(edge_index[1], dtype=np.int64)
    batch = np.asarray(batch, dtype=np.int64)

    # real node id -> padded id
    s_pad = (src // NLR) * NLOC + src % NLR
    d_pad = (dst // NLR) * NLOC + dst % NLR

    ecore = d_pad // NLOC
    iloc = (d_pad % NLOC) // P          # dst tile within core
    drel = (d_pad % P).astype(np.float32)
    s_rank = s_pad // NLOC
    s_loc = s_pad % NLOC
    seg = (s_loc // SUBSH).astype(np.int64)
    srel = (s_rank * SUBSH + s_loc % SUBSH).astype(np.int64)

    # idx bucket order: (group, seg, tile-in-group) so one gather call's
    # buckets are contiguous
    grp = iloc // GT
    ti = iloc % GT
    b_core = (grp * NSEG + seg) * GT + ti
    nb = T * NSEG
    bucket = ecore * nb + b_core
    # sort by (bucket, src) so gather reads within a bucket are
    # ascending in the table -> better HBM row locality
    order = np.argsort(bucket * (1 << 16) + srel, kind="stable")
    counts = np.bincount(bucket, minlength=CORES * nb)
    starts = np.zeros(CORES * nb + 1, dtype=np.int64)
    np.cumsum(counts, out=starts[1:])
    pos = np.arange(len(src)) - starts[bucket[order]]

    # per-bucket chunk count = max over cores, >= 1
    cc = counts.reshape(CORES, nb)
    ch_b = np.maximum(1, (cc.max(axis=0) + P - 1) // P)  # [T*NSEG]
    cap_b = ch_b * P
    cap_off = np.zeros(nb + 1, dtype=np.int64)
    np.cumsum(cap_b, out=cap_off[1:])
    CAPSUM = int(cap_off[-1])

    idx_cap = np.zeros((CORES, CAPSUM), dtype=np.int16)
    drel_cap = np.full((CORES, CAPSUM), 200.0, dtype=np.float32)
    bs = bucket[order]
    bc = bs // nb
    bl = bs % nb
    flatpos = cap_off[bl] + pos
    idx_cap[bc, flatpos] = srel[order].astype(np.int16)
    drel_cap[bc, flatpos] = drel[order]

    # pack idx: bucket b occupies cols [cap_off[b]//16, cap_off[b+1]//16)
    idx_packed = np.zeros((CORES, P, CAPSUM // 16), dtype=np.int16)
    v16 = idx_cap.reshape(CORES, CAPSUM // 16, 16)
    idx_packed[:] = np.tile(v16.transpose(0, 2, 1), (1, 8, 1))

    # dstrel packed in PER-TILE order (tile-major, segs concatenated) so
    # the S-matrix for a whole tile builds in one DVE op; [p, j] =
    # drel of that tile's j-th chunk, edge j*128+p.
    chunk_src = []  # bucket index per chunk, in dr order
    for i in range(T):
        g_, t_ = i // GT, i % GT
        for s in range(NSEG):
            b = (g_ * NSEG + s) * GT + t_
            chunk_src.extend([b] * int(ch_b[b]))
    chunk_src = np.asarray(chunk_src, dtype=np.int64)
    CHSUM = int(ch_b.sum())
    assert len(chunk_src) == CHSUM
    nth = np.zeros(nb, dtype=np.int64)
    firstrow = np.zeros(CHSUM, dtype=np.int64)
    for j in range(CHSUM):
        b = chunk_src[j]
        firstrow[j] = cap_off[b] + nth[b] * P
        nth[b] += 1
    rows = firstrow[:, None] + np.arange(P)[None, :]     # [CHSUM, 128]
    dr_packed = drel_cap[:, rows]                        # [C, CHSUM, 128]
    dr_packed = np.ascontiguousarray(
        dr_packed.transpose(0, 2, 1)).astype(MSG_NP)     # [C, 128, CHSUM]

    # x per core, padded; fp16 with trailing ones column for counts
    x_loc = np.zeros((CORES, NLOC, IN_DIM), dtype=np.float32)
    x_loc[:, :NLR] = x.reshape(CORES, NLR, IN_DIM)
    xr = x_loc.reshape(CORES, T, P, IN_DIM)
    xaug = np.ones((CORES, T, P, IN_DIM + 1), dtype=np.float32)
    xaug[..., :IN_DIM] = xr
    xaug = xaug.astype(MSG_NP)

    # h0^T resident layout: [128, T*256]; cols t*256 + k*128 + n
    h0t = np.zeros((CORES, P, T, 2, P), dtype=np.float32)
    h0t[:, :, :, 0, :] = xr.transpose(0, 3, 1, 2)
    h0t = h0t.reshape(CORES, P, T * HID).astype(MSG_NP)

    # batch rel (f32 for tensor_scalar is_equal), pads get 200
    b_loc = np.full((CORES, NLOC), 200.0, dtype=np.float32)
    b_loc[:, :NLR] = batch.reshape(CORES, NLR).astype(np.float32)
    batchrel = np.ascontiguousarray(
        b_loc.reshape(CORES, T, P).transpose(0, 2, 1))   # [C, 128, T]

    arrays = dict(
        idx=idx_packed, dstrel=dr_packed, xaug=xaug, h0t=h0t,
        batchrel=batchrel,
    )
    meta = dict(cfg=c, ch_b=ch_b, cap_off=cap_off, CAPSUM=CAPSUM,
                CHSUM=CHSUM)
    return arrays, meta


def shared_arrays(ggnn_weight, w_ih, w_hh, b_ih, b_hh, steps):
    f16 = MSG_NP
    gg = np.asarray(ggnn_weight, np.float32).reshape(steps, 2, P, HID)
    w_all = np.ascontiguousarray(
        gg.transpose(2, 0, 1, 3).reshape(P, steps * 2 * HID))
    # wihT[f, (k*6+c)*128+g] = w_ih[c*128+g, k*128+f]
    wih = np.asarray(w_ih, np.float32).reshape(6, P, 2, P)
    wihT = np.ascontiguousarray(wih.transpose(3, 2, 0, 1).reshape(P, 12 * P))
    whh = np.asarray(w_hh, np.float32).reshape(6, P, 2, P)
    whhT = np.ascontiguousarray(whh.transpose(3, 2, 0, 1).reshape(P, 12 * P))
    b_ih = np.asarray(b_ih, np.float32)
    b_hh = np.asarray(b_hh, np.float32)
    brz = (b_ih + b_hh)[:512].reshape(4, P).T          # [128, 4]
    bin_ = b_ih[512:].reshape(2, P).T                  # [128, 2]
    bhn = b_hh[512:].reshape(2, P).T                   # [128, 2]
    iota = np.tile(np.arange(P, dtype=np.float32), (P, 1))
    return dict(
        wihT=wihT.astype(f16), whhT=whhT.astype(f16),
        w_all=w_all.astype(f16),
        b_rz=np.ascontiguousarray(brz), b_in=np.ascontiguousarray(bin_),
        b_hn=np.ascontiguousarray(bhn),
        iota_f16=iota.astype(f16),
        iota_f32=iota,
        ident_f16=np.eye(P, dtype=np.float32).astype(f16),
    )


def build(meta):
    c = meta["cfg"]
    T, NSEG, SEGROWS, NLOC, STEPS, GT, NGRP = (
        c["TILES"], c["NSEG"], c["SEGROWS"], c["NLOC"], c["STEPS"],
        c["GT"], c["NGRP"])
    GCH = c["GCHUNK"]
    SUBSH = c["SUBSH"]
    TPS = T // NSEG  # tiles per sub-shard
    ch_b = meta["ch_b"]
    cap_off = meta["cap_off"]
    CAPSUM, CHSUM = meta["CAPSUM"], meta["CHSUM"]

    def bidx(g, s, t_):
        return (g * NSEG + s) * GT + t_

    # per-tile chunk count and per-tile dr col offsets (dr is tile-major)
    chT = np.array([int(sum(ch_b[bidx(i // GT, s, i % GT)]
                            for s in range(NSEG))) for i in range(T)])
    chT_off = np.zeros(T + 1, dtype=np.int64)
    np.cumsum(chT, out=chT_off[1:])
    CTMAX = int(chT.max())
    GMAX = max(int(sum(ch_b[bidx(g, s, t_)] for t_ in range(GT)))
               for g in range(NGRP) for s in range(NSEG))

    nc = bacc.Bacc("TRN2", target_bir_lowering=False, debug=False,
                   num_devices=CORES, num_swdge_queues=4,
                   dynamic_dma_scratch_size=32768)

    idx_d = nc.dram_tensor("idx", [P, CAPSUM // 16], I16, kind="ExternalInput")
    dr_d = nc.dram_tensor("dstrel", [P, CHSUM], F16, kind="ExternalInput")
    xaug_d = nc.dram_tensor("xaug", [T, P, IN_DIM + 1], F16,
                            kind="ExternalInput")
    h0t_d = nc.dram_tensor("h0t", [P, T * HID], F16, kind="ExternalInput")
    batch_d = nc.dram_tensor("batchrel", [P, T], F32, kind="ExternalInput")
    wall_d = nc.dram_tensor("w_all", [P, STEPS * 2 * HID], F16,
                            kind="ExternalInput")
    wih_d = nc.dram_tensor("wihT", [P, 12 * P], F16, kind="ExternalInput")
    whh_d = nc.dram_tensor("whhT", [P, 12 * P], F16, kind="ExternalInput")
    brz_d = nc.dram_tensor("b_rz", [P, 4], F32, kind="ExternalInput")
    bin_d = nc.dram_tensor("b_in", [P, 2], F32, kind="ExternalInput")
    bhn_d = nc.dram_tensor("b_hn", [P, 2], F32, kind="ExternalInput")
    iotah_d = nc.dram_tensor("iota_f16", [P, P], F16, kind="ExternalInput")
    iotaf_d = nc.dram_tensor("iota_f32", [P, P], F32, kind="ExternalInput")
    identh_d = nc.dram_tensor("ident_f16", [P, P], F16, kind="ExternalInput")
    pool_d = nc.dram_tensor("pool", [P, IN_DIM + 1 + HID], F32,
                            kind="ExternalOutput")

    with tile.TileContext(nc) as tc:
        with (
            tc.tile_pool(name="const", bufs=1) as cpool,
            tc.tile_pool(name="stream", bufs=4) as stpool,
            tc.tile_pool(name="gq", bufs=3) as gpool,
            tc.tile_pool(name="Sp", bufs=3) as spool,
            tc.tile_pool(name="work", bufs=3) as wpool,
            tc.tile_pool(name="dram", bufs=1, space="DRAM") as dpool,
        ):
            nc.gpsimd.load_library(library_config.mlp)

            def load_const(name, dram, shape, dtype):
                t_ = cpool.tile(shape, dtype, name=name)
                nc.sync.dma_start(out=t_[:], in_=dram)
                return t_

            wih_sb = load_const("wih", wih_d[:, :], [P, 12 * P], F16)
            whh_sb = load_const("whh", whh_d[:, :], [P, 12 * P], F16)
            wall_sb = load_const("wall", wall_d[:, :], [P, STEPS * 2 * HID],
                                 F16)
            brz_sb = load_const("brz", brz_d[:, :], [P, 4], F32)
            bin_sb = load_const("bin", bin_d[:, :], [P, 2], F32)
            bhn_sb = load_const("bhn", bhn_d[:, :], [P, 2], F32)
            iotah_sb = load_const("iotah", iotah_d[:, :], [P, P], F16)
            iotaf_sb = load_const("iotaf", iotaf_d[:, :], [P, P], F32)
            identh_sb = load_const("identh", identh_d[:, :], [P, P], F16)
            batch_sb = load_const("batch", batch_d[:, :], [P, T], F32)

            hT = []
            for i in range(T):
                t_ = cpool.tile([P, HID], F16, name=f"hT{i}")
                nc.sync.dma_start(out=t_[:], in_=h0t_d[:, i * HID:(i + 1) * HID])
                hT.append(t_)

            swdge_ctr = [0]

            cc_ins = [[dpool.tile([SUBSH, HID], F16, name=f"cc_in{t}_{k}")
                       for k in range(NSEG)] for t in range(STEPS)]
            cc_outs = [[dpool.tile([SEGROWS, HID], F16,
                                   name=f"cc_out{t}_{k}", addr_space="Shared")
                        for k in range(NSEG)] for t in range(STEPS)]

            with (
                tc.tile_pool(name="accps", bufs=4, space="PSUM") as accps,
                tc.tile_pool(name="rzps", bufs=2, space="PSUM") as rzps,
                tc.tile_pool(name="nps", bufs=2, space="PSUM") as nps,
            ):
                def gru_tile(t, i, agg_ps):
                    """GRU for dst tile i at step t; aggT in PSUM
                    [feat, (k, node)] layout."""
                    aggT_sb = wpool.tile([P, HID], F16, tag="aggT",
                                         name=f"aT{t}_{i}")
                    nc.scalar.copy(out=aggT_sb[:], in_=agg_ps[:])

                    rz_ps = rzps.tile([P, 512], F32, tag="rz",
                                      name=f"rz{t}_{i}")
                    for cch in range(4):
                        o = rz_ps[:, cch * P:(cch + 1) * P]
                        for k in range(2):
                            nc.tensor.matmul(
                                out=o,
                                lhsT=wih_sb[:, (k * 6 + cch) * P:
                                            (k * 6 + cch + 1) * P],
                                rhs=aggT_sb[:, k * P:(k + 1) * P],
                                start=(cch == 0 and k == 0), stop=False,
                                skip_group_check=True)
                        for k in range(2):
                            nc.tensor.matmul(
                                out=o,
                                lhsT=whh_sb[:, (k * 6 + cch) * P:
                                            (k * 6 + cch + 1) * P],
                                rhs=hT[i][:, k * P:(k + 1) * P],
                                start=False, stop=(cch == 3 and k == 1),
                                skip_group_check=True)
                    # rz + bias (broadcast over nodes), then sigmoid
                    rzb = wpool.tile([P, 512], F16, tag="rzb",
                                     name=f"rzb{t}_{i}")
                    nc.vector.tensor_tensor(
                        out=rzb[:].rearrange("p (c n) -> p c n", n=P),
                        in0=rz_ps[:].rearrange("p (c n) -> p c n", n=P),
                        in1=brz_sb[:].rearrange("p (c o) -> p c o", o=1)
                        .to_broadcast([P, 4, P]),
                        op=mybir.AluOpType.add)
                    rz_sb = wpool.tile([P, 512], F16, tag="rzsb",
                                       name=f"rzs{t}_{i}")
                    nc.scalar.activation(
                        out=rz_sb[:], in_=rzb[:],
                        func=mybir.ActivationFunctionType.Sigmoid)

                    inhn_ps = nps.tile([P, 512], F32, tag="n",
                                       name=f"n{t}_{i}")
                    for cch in range(2):
                        o = inhn_ps[:, cch * P:(cch + 1) * P]
                        for k in range(2):
                            nc.tensor.matmul(
                                out=o,
                                lhsT=wih_sb[:, (k * 6 + 4 + cch) * P:
                                            (k * 6 + 4 + cch + 1) * P],
                                rhs=aggT_sb[:, k * P:(k + 1) * P],
                                start=(cch == 0 and k == 0), stop=False,
                                skip_group_check=True)
                    for cch in range(2):
                        o = inhn_ps[:, 256 + cch * P:256 + (cch + 1) * P]
                        for k in range(2):
                            nc.tensor.matmul(
                                out=o,
                                lhsT=whh_sb[:, (k * 6 + 4 + cch) * P:
                                            (k * 6 + 4 + cch + 1) * P],
                                rhs=hT[i][:, k * P:(k + 1) * P],
                                start=False,
                                stop=(cch == 1 and k == 1),
                                skip_group_check=True)
                    # hnb = h_n + b_hn ; inb = i_n + b_in
                    hnb = wpool.tile([P, HID], F16, tag="tmp",
                                     name=f"hb{t}_{i}", bufs=6)
                    nc.vector.tensor_tensor(
                        out=hnb[:].rearrange("p (c n) -> p c n", n=P),
                        in0=inhn_ps[:, 256:512]
                        .rearrange("p (c n) -> p c n", n=P),
                        in1=bhn_sb[:].rearrange("p (c o) -> p c o", o=1)
                        .to_broadcast([P, 2, P]),
                        op=mybir.AluOpType.add)
                    inb = wpool.tile([P, HID], F16, tag="tmp",
                                     name=f"ib{t}_{i}", bufs=6)
                    nc.vector.tensor_tensor(
                        out=inb[:].rearrange("p (c n) -> p c n", n=P),
                        in0=inhn_ps[:, 0:256]
                        .rearrange("p (c n) -> p c n", n=P),
                        in1=bin_sb[:].rearrange("p (c o) -> p c o", o=1)
                        .to_broadcast([P, 2, P]),
                        op=mybir.AluOpType.add)
                    t1 = wpool.tile([P, HID], F16, tag="tmp",
                                    name=f"t1{t}_{i}", bufs=6)
                    nc.vector.tensor_mul(out=t1[:], in0=rz_sb[:, 0:HID],
                                         in1=hnb[:])
                    t2 = wpool.tile([P, HID], F16, tag="tmp",
                                    name=f"t2{t}_{i}", bufs=6)
                    nc.vector.tensor_add(out=t2[:], in0=t1[:], in1=inb[:])
                    n_sb = wpool.tile([P, HID], F16, tag="tmp",
                                      name=f"ns{t}_{i}", bufs=6)
                    nc.scalar.activation(
                        out=n_sb[:], in_=t2[:],
                        func=mybir.ActivationFunctionType.Tanh)
                    d_sb = wpool.tile([P, HID], F16, tag="tmp",
                                      name=f"d{t}_{i}", bufs=6)
                    nc.vector.tensor_sub(out=d_sb[:], in0=hT[i][:],
                                         in1=n_sb[:])
                    zd = wpool.tile([P, HID], F16, tag="tmp",
                                    name=f"zd{t}_{i}", bufs=6)
                    nc.vector.tensor_mul(out=zd[:], in0=rz_sb[:, HID:512],
                                         in1=d_sb[:])
                    nc.vector.tensor_add(out=hT[i][:], in0=zd[:], in1=n_sb[:])

                def scatter_group(t, g):
                    """Gathers + S-build + transposed scatter for group g.
                    Returns the GT aggT PSUM tiles."""
                    i0 = g * GT
                    dr0 = int(chT_off[i0])
                    dr1 = int(chT_off[i0 + GT])
                    dr_sb = stpool.tile([P, GT * CTMAX], F16, tag="dr",
                                        name=f"dr{t}_{g}")
                    nc.sync.dma_start(out=dr_sb[:, 0:dr1 - dr0],
                                      in_=dr_d[:, dr0:dr1])

                    Gs = []
                    for s in range(NSEG):
                        b0 = bidx(g, s, 0)
                        ic0 = int(cap_off[b0] // 16)
                        ic1 = int(cap_off[b0 + GT] // 16)
                        nch = (int(cap_off[b0 + GT]) - int(cap_off[b0])) // P
                        idx_sb = stpool.tile([P, GMAX * 8], I16, tag="idx",
                                             name=f"ix{t}_{g}_{s}")
                        nc.sync.dma_start(out=idx_sb[:, 0:ic1 - ic0],
                                          in_=idx_d[:, ic0:ic1])
                        G = gpool.tile([P, GMAX * HID], F16, tag="g",
                                       name=f"g{t}_{g}_{s}")
                        c0 = 0
                        while c0 < nch:
                            nsub = min(GCH, nch - c0)
                            nc.gpsimd.dma_gather(
                                G[:, c0 * HID:(c0 + nsub) * HID]
                                .rearrange("p (j d) -> p j d", d=HID),
                                cc_outs[t][s][:, :],
                                idx_sb[:, c0 * 8:(c0 + nsub) * 8],
                                nsub * P, nsub * P, HID,
                                queue_num=swdge_ctr[0] % 4)
                            swdge_ctr[0] += 1
                            c0 += nsub
                        Gs.append(G)

                    # one S build per tile (dr is tile-major)
                    S_tiles = []
                    for t_ in range(GT):
                        i = i0 + t_
                        cs = int(chT[i])
                        S_sb = spool.tile([P, CTMAX * P], F16, tag="S",
                                          name=f"S{t}_{g}_{t_}")
                        bdr = int(chT_off[i]) - dr0
                        nc.vector.tensor_tensor(
                            out=S_sb[:, :cs * P]
                            .rearrange("p (j d) -> p j d", d=P),
                            in0=dr_sb[:, bdr:bdr + cs]
                            .to_broadcast([P, cs, P]),
                            in1=iotah_sb[:]
                            .rearrange("p (a b) -> p a b", a=1)
                            .to_broadcast([P, cs, P]),
                            op=mybir.AluOpType.is_equal)
                        S_tiles.append(S_sb)

                    aggs = []
                    for t_ in range(GT):
                        i = i0 + t_
                        agg = accps.tile([P, HID], F32, tag="acc",
                                         name=f"ag{t}_{g}_{t_}")
                        aggs.append(agg)
                        nchunks = int(chT[i])
                        done = 0
                        scol = 0
                        for s in range(NSEG):
                            b = bidx(g, s, t_)
                            cs = int(ch_b[b])
                            joff = (int(cap_off[b]) -
                                    int(cap_off[bidx(g, s, 0)])) // P
                            for j in range(cs):
                                gcol = (joff + j) * HID
                                for k in range(2):
                                    nc.tensor.matmul(
                                        out=agg[:, k * P:(k + 1) * P],
                                        lhsT=Gs[s][:, gcol + k * P:
                                                   gcol + (k + 1) * P],
                                        rhs=S_tiles[t_][:, (scol + j) * P:
                                                        (scol + j + 1) * P],
                                        start=(done == 0 and k == 0),
                                        stop=(done == nchunks - 1 and k == 1),
                                        skip_group_check=True)
                                done += 1
                            scol += cs
                    return aggs

                for t in range(STEPS):
                    # ---- phase M: messages + AllGather ----
                    woff = t * 2 * HID
                    for i in range(T):
                        m_ps = accps.tile([P, HID], F32, tag="acc",
                                          name=f"mp{t}_{i}")
                        nc.tensor.matmul(
                            out=m_ps[:], lhsT=hT[i][:, 0:P],
                            rhs=wall_sb[:, woff:woff + HID],
                            start=True, stop=False)
                        nc.tensor.matmul(
                            out=m_ps[:], lhsT=hT[i][:, P:HID],
                            rhs=wall_sb[:, woff + HID:woff + 2 * HID],
                            start=False, stop=True)
                        m_sb = wpool.tile([P, HID], F16, tag="m",
                                          name=f"m{t}_{i}")
                        nc.scalar.copy(out=m_sb[:], in_=m_ps[:])
                        k = i // TPS
                        ioff = (i % TPS) * P
                        nc.sync.dma_start(
                            out=cc_ins[t][k][ioff:ioff + P, :], in_=m_sb[:])
                        if i % TPS == TPS - 1:
                            nc.gpsimd.collective_compute(
                                "AllGather", mybir.AluOpType.bypass,
                                replica_groups=[list(range(CORES))],
                                ins=[cc_ins[t][k].opt()],
                                outs=[cc_outs[t][k].opt()])

                    # ---- phase A: software-pipelined groups ----
                    prev = None
                    for g in range(NGRP):
                        aggs = scatter_group(t, g)
                        if prev is not None:
                            pg, paggs = prev
                            for t_ in range(GT):
                                gru_tile(t, pg * GT + t_, paggs[t_])
                        prev = (g, aggs)
                    pg, paggs = prev
                    for t_ in range(GT):
                        gru_tile(t, pg * GT + t_, paggs[t_])

            # ---- phase C: per-graph pooling partials ----
            with (
                tc.tile_pool(name="poolps", bufs=1, space="PSUM") as ppsC,
                tc.tile_pool(name="hfps", bufs=2, space="PSUM") as hfps,
            ):
                px_ps = ppsC.tile([P, IN_DIM + 1], F32, name="px")
                ph_ps = ppsC.tile([P, HID], F32, name="ph")
                for i in range(T):
                    hf_ps = hfps.tile([P, HID], F16, tag="hf", name=f"hf{i}")
                    nc.tensor.transpose(out=hf_ps[:, 0:P],
                                        in_=hT[i][:, 0:P],
                                        identity=identh_sb[:])
                    nc.tensor.transpose(out=hf_ps[:, P:HID],
                                        in_=hT[i][:, P:HID],
                                        identity=identh_sb[:])
                    hfin_sb = wpool.tile([P, HID], F16, tag="hfin",
                                         name=f"hfs{i}")
                    nc.scalar.copy(out=hfin_sb[:], in_=hf_ps[:])
                    sg_sb = wpool.tile([P, P], F16, tag="sg", name=f"sg{i}")
                    nc.vector.tensor_scalar(
                        out=sg_sb[:], in0=iotaf_sb[:],
                        scalar1=batch_sb[:, i:i + 1], scalar2=None,
                        op0=mybir.AluOpType.is_equal)
                    xo_sb = stpool.tile([P, IN_DIM + 1], F16, tag="xo",
                                        name=f"xo{i}")
                    nc.sync.dma_start(out=xo_sb[:], in_=xaug_d[i, :, :])
                    nc.tensor.matmul(out=px_ps[:], lhsT=sg_sb[:], rhs=xo_sb[:],
                                     start=(i == 0), stop=(i == T - 1))
                    nc.tensor.matmul(out=ph_ps[:], lhsT=sg_sb[:],
                                     rhs=hfin_sb[:],
                                     start=(i == 0), stop=(i == T - 1))
                px_sb = wpool.tile([P, IN_DIM + 1], F32, name="pxs")
                nc.vector.tensor_copy(out=px_sb[:], in_=px_ps[:])
                ph_sb = wpool.tile([P, HID], F32, name="phs")
                nc.vector.tensor_copy(out=ph_sb[:], in_=ph_ps[:])
                nc.sync.dma_start(out=pool_d[:, 0:IN_DIM + 1], in_=px_sb[:])
                nc.sync.dma_start(out=pool_d[:, IN_DIM + 1:], in_=ph_sb[:])

    nc.compile()
    return nc


_CACHE = {}


def run_device(inputs, cfg, trace=False, tmpdir=None):
    """Preprocess, compile (cached), run on 8 cores; returns per-core pool
    partials [CORES, 128, 385] plus the BassKernelResults."""
    c = _derived(cfg)
    arrays, meta = preprocess(
        inputs["x"], inputs["edge_index"], inputs["batch"], cfg)
    shared = shared_arrays(
        inputs["ggnn_weight"], inputs["w_ih"], inputs["w_hh"],
        inputs["b_ih"], inputs["b_hh"], c["STEPS"])

    key = (tuple(meta["ch_b"].tolist()), c["STEPS"], c["NLOC"], c["GT"],
           c["GCHUNK"])
    if key not in _CACHE:
        _CACHE[key] = build(meta)
    nc = _CACHE[key]

    in_maps = []
    for core in range(CORES):
        m = {k: np.ascontiguousarray(v[core]) for k, v in arrays.items()}
        m.update(shared)
        in_maps.append(m)
    kw = {}
    if trace:
        kw = dict(trace=True, tmpdir=tmpdir)
    res = bass_utils.run_bass_kernel_spmd(
        nc, in_maps, core_ids=list(range(CORES)), **kw)
    pool = np.stack([res.results[cr]["pool"] for cr in range(CORES)])
    return pool, res


def host_epilogue(pool, inputs):
    """Sum per-core partials, mean-pool, and run the classifier MLP."""
    tot = pool.sum(axis=0, dtype=np.float64).astype(np.float32)
    xsum = tot[:, :IN_DIM]
    cnt = tot[:, IN_DIM]
    hsum = tot[:, IN_DIM + 1:]
    feat = np.concatenate([xsum, hsum], axis=1)
    pooled = feat / np.maximum(cnt, 1.0)[:, None]
    w1 = np.asarray(inputs["mlp_w1"], np.float32)
    b1 = np.asarray(inputs["mlp_b1"], np.float32)
    w2 = np.asarray(inputs["mlp_w2"], np.float32)
    b2 = np.asarray(inputs["mlp_b2"], np.float32)
    hdn = np.maximum(pooled @ w1.T + b1, 0.0)
    return (hdn @ w2.T + b2).astype(np.float32)


def kernel(**inputs):
    cfg = _default_cfg()
    pool, _ = run_device(inputs, cfg)
    return host_epilogue(pool, inputs)



# revision 13
# speedup vs baseline: 1.1708x; 1.1622x over previous
"""DevignModel (GGNN message passing) Trainium2 kernel, 8 NeuronCores.

Strategy (graph/edge-cut parallelism, v2):
  - Nodes sharded contiguously across 8 cores (12800 padded rows each);
    h kept feature-major (h^T, fp16) resident in SBUF, grouped 2 tiles
    per SBUF allocation for 256-wide GRU matmul streams.
  - Linearity trick: segment_sum(m[src]) = segment_sum(h[src]) @ W_t, so
    the kernel aggregates h directly and applies F_t = W_t @ w_ih^T
    (host-precomputed, fp16) after aggregation.  No per-step message
    matmul, and the step-0 table is just x (host-provided, 128-wide
    gather rows, no step-0 collective).
  - Per step: gather h rows per edge from the (allgathered) node-major
    table via dma_gather (4 SWDGE queues = 4 Q7 core-pairs generating
    descriptors concurrently); S one-hot matrices built on DVE; scatter
    computed node-major per dst tile: agg[d, f] += S_chunk^T @ G_chunk
    (S stationary: 1 LDWEIGHTS + 1 256-col matmul per chunk), then
    PE-transposed to feature-major aggT for the GRU.
  - GRU in transposed [gate, (k, tile, node)] layout at 2-tile group
    granularity; biases fused into scalar-engine activations.
  - After each group's GRU the updated h tiles are PE-transposed to
    node-major and DMA'd to the next step's bounce buffer; each
    sub-shard's AllGather launches as soon as its 20 tiles are done, so
    collectives overlap the remaining scatter work of the same step.
  - Epilogue: per-graph sums via one-hot matmul into PSUM; host sums the
    8 partial [128,385] blocks and runs the tiny MLP.
"""

import numpy as np

import concourse.bass as bass
import concourse.bacc as bacc
import concourse.mybir as mybir
import concourse.tile as tile
from concourse import bass_utils, library_config

F32 = mybir.dt.float32
F16 = mybir.dt.float16
I16 = mybir.dt.int16

CORES = 8
P = 128
HID = 256
IN_DIM = 128

MSG_NP = np.float16


def _default_cfg():
    return dict(
        NREAL=100000,
        E=3200000,
        STEPS=6,
        NGRAPH=128,
        NLOC=12800,  # padded nodes per core (multiple of 128)
        NSEG=5,      # src segments = sub-shard AllGather pieces
        GT=2,        # dst tiles per gather group
        GCHUNK=8,    # max 128-row chunks per dma_gather call
    )


def _derived(cfg):
    c = dict(cfg)
    c["NLOCREAL"] = c["NREAL"] // CORES
    c["TILES"] = c["NLOC"] // P
    c["NPAD"] = c["NLOC"] * CORES
    c["SEGROWS"] = c["NPAD"] // c["NSEG"]
    c["SUBSH"] = c["NLOC"] // c["NSEG"]
    c["NGRP"] = c["TILES"] // c["GT"]
    assert c["SEGROWS"] <= 32768
    assert c["NLOC"] % P == 0 and c["NLOC"] % c["NSEG"] == 0
    assert c["SUBSH"] % P == 0 and c["TILES"] % c["NSEG"] == 0
    assert c["TILES"] % c["GT"] == 0
    return c


def preprocess(x, edge_index, batch, cfg):
    """Build per-core and shared device arrays. Returns (arrays, shared, meta)."""
    c = _derived(cfg)
    NLR, NLOC, T, NSEG, GT = (
        c["NLOCREAL"], c["NLOC"], c["TILES"], c["NSEG"], c["GT"])
    SUBSH = c["SUBSH"]
    SEGROWS = c["SEGROWS"]
    NPAD = c["NPAD"]

    x = np.asarray(x, dtype=np.float32)
    src = np.asarray(edge_index[0], dtype=np.int64)
    dst = np.asarray(edge_index[1], dtype=np.int64)
    batch = np.asarray(batch, dtype=np.int64)

    # real node id -> padded id
    s_pad = (src // NLR) * NLOC + src % NLR
    d_pad = (dst // NLR) * NLOC + dst % NLR

    ecore = d_pad // NLOC
    iloc = (d_pad % NLOC) // P          # dst tile within core
    drel = (d_pad % P).astype(np.float32)
    s_rank = s_pad // NLOC
    s_loc = s_pad % NLOC
    seg = (s_loc // SUBSH).astype(np.int64)
    srel = (s_rank * SUBSH + s_loc % SUBSH).astype(np.int64)

    # idx bucket order: (group, seg, tile-in-group) so one gather call's
    # buckets are contiguous
    grp = iloc // GT
    ti = iloc % GT
    b_core = (grp * NSEG + seg) * GT + ti
    nb = T * NSEG
    bucket = ecore * nb + b_core
    # sort by (bucket, src) so gather reads within a bucket are
    # ascending in the table -> better HBM row locality
    order = np.argsort(bucket * (1 << 16) + srel, kind="stable")
    counts = np.bincount(bucket, minlength=CORES * nb)
    starts = np.zeros(CORES * nb + 1, dtype=np.int64)
    np.cumsum(counts, out=starts[1:])
    pos = np.arange(len(src)) - starts[bucket[order]]

    # per-bucket chunk count = max over cores, >= 1
    cc = counts.reshape(CORES, nb)
    ch_b = np.maximum(1, (cc.max(axis=0) + P - 1) // P)  # [T*NSEG]
    cap_b = ch_b * P
    cap_off = np.zeros(nb + 1, dtype=np.int64)
    np.cumsum(cap_b, out=cap_off[1:])
    CAPSUM = int(cap_off[-1])

    idx_cap = np.zeros((CORES, CAPSUM), dtype=np.int16)
    drel_cap = np.full((CORES, CAPSUM), 200.0, dtype=np.float32)
    bs = bucket[order]
    bc = bs // nb
    bl = bs % nb
    flatpos = cap_off[bl] + pos
    idx_cap[bc, flatpos] = srel[order].astype(np.int16)
    drel_cap[bc, flatpos] = drel[order]

    # pack idx: bucket b occupies cols [cap_off[b]//16, cap_off[b+1]//16)
    idx_packed = np.zeros((CORES, P, CAPSUM // 16), dtype=np.int16)
    v16 = idx_cap.reshape(CORES, CAPSUM // 16, 16)
    idx_packed[:] = np.tile(v16.transpose(0, 2, 1), (1, 8, 1))

    # dstrel packed in PER-TILE order (tile-major, segs concatenated) so
    # the S-matrix for a whole tile builds in one DVE op; [p, j] =
    # drel of that tile's j-th chunk, edge j*128+p.
    chunk_src = []  # bucket index per chunk, in dr order
    for i in range(T):
        g_, t_ = i // GT, i % GT
        for s in range(NSEG):
            b = (g_ * NSEG + s) * GT + t_
            chunk_src.extend([b] * int(ch_b[b]))
    chunk_src = np.asarray(chunk_src, dtype=np.int64)
    CHSUM = int(ch_b.sum())
    assert len(chunk_src) == CHSUM
    nth = np.zeros(nb, dtype=np.int64)
    firstrow = np.zeros(CHSUM, dtype=np.int64)
    for j in range(CHSUM):
        b = chunk_src[j]
        firstrow[j] = cap_off[b] + nth[b] * P
        nth[b] += 1
    rows = firstrow[:, None] + np.arange(P)[None, :]     # [CHSUM, 128]
    dr_packed = drel_cap[:, rows]                        # [C, CHSUM, 128]
    dr_packed = np.ascontiguousarray(
        dr_packed.transpose(0, 2, 1)).astype(MSG_NP)     # [C, 128, CHSUM]

    # x per core, padded; fp16 with trailing ones column for counts
    x_loc = np.zeros((CORES, NLOC, IN_DIM), dtype=np.float32)
    x_loc[:, :NLR] = x.reshape(CORES, NLR, IN_DIM)
    xr = x_loc.reshape(CORES, T, P, IN_DIM)
    xaug = np.ones((CORES, T, P, IN_DIM + 1), dtype=np.float32)
    xaug[..., :IN_DIM] = xr
    xaug = xaug.astype(MSG_NP)

    # h0^T resident layout, 2-tile groups: [128, NGRP*512];
    # col g*512 + k*256 + tt*128 + n  (feature f = k*128 + p)
    NGRP = T // GT
    h0t = np.zeros((CORES, P, NGRP, 2, GT, P), dtype=np.float32)
    # xr [C, tile, node, feat] -> feat-major per (group, tt)
    xr_t = xr.transpose(0, 3, 1, 2).reshape(CORES, P, NGRP, GT, P)
    h0t[:, :, :, 0, :, :] = xr_t
    h0t = h0t.reshape(CORES, P, NGRP * 2 * GT * P).astype(MSG_NP)

    # step-0 gather table: full padded x, node-major, in AllGather row
    # order: segment-major, then (core, local-within-subshard) like the
    # cc_out buffers, so srel indexes it directly.
    x_all = np.zeros((NPAD, IN_DIM), dtype=np.float32)
    x_all_r = x_all.reshape(CORES, NLOC, IN_DIM)
    x_all_r[:, :NLR] = x.reshape(CORES, NLR, IN_DIM)
    h0seg = np.ascontiguousarray(
        x_all.reshape(CORES, NSEG, SUBSH, IN_DIM)
        .transpose(1, 0, 2, 3)).reshape(NSEG * SEGROWS, IN_DIM).astype(MSG_NP)

    # batch rel (f32 for tensor_scalar is_equal), pads get 200
    b_loc = np.full((CORES, NLOC), 200.0, dtype=np.float32)
    b_loc[:, :NLR] = batch.reshape(CORES, NLR).astype(np.float32)
    batchrel = np.ascontiguousarray(
        b_loc.reshape(CORES, T, P).transpose(0, 2, 1))   # [C, 128, T]

    arrays = dict(
        idx=idx_packed, dstrel=dr_packed, xaug=xaug, h0t=h0t,
        batchrel=batchrel,
    )
    shared0 = dict(h0seg=h0seg)
    meta = dict(cfg=c, ch_b=ch_b, cap_off=cap_off, CAPSUM=CAPSUM,
                CHSUM=CHSUM)
    return arrays, shared0, meta


def shared_arrays(ggnn_weight, w_ih, w_hh, b_ih, b_hh, steps):
    f16 = MSG_NP
    gg = np.asarray(ggnn_weight, np.float64)                 # [S, 256, 256]
    wih = np.asarray(w_ih, np.float64)                       # [768, 256]
    # F_t = W_t @ w_ih.T  [256, 768]
    F = np.einsum("sac,bc->sab", gg, wih).astype(np.float32)
    # FT[f, ((t*2+k)*6+c)*128+g] = F_t[k*128+f, c*128+g]
    Fr = F.reshape(steps, 2, P, 6, P)                        # [t,k,p,c,g]
    FT = np.ascontiguousarray(
        Fr.transpose(2, 0, 1, 3, 4).reshape(P, steps * 12 * P))
    whh = np.asarray(w_hh, np.float32).reshape(6, P, 2, P)
    whhT = np.ascontiguousarray(whh.transpose(3, 2, 0, 1).reshape(P, 12 * P))
    b_ih = np.asarray(b_ih, np.float32)
    b_hh = np.asarray(b_hh, np.float32)
    brz = (b_ih + b_hh)[:512].reshape(4, P).T          # [128, 4]
    bin_ = b_ih[512:].reshape(2, P).T                  # [128, 2]
    bhn = b_hh[512:].reshape(2, P).T                   # [128, 2]
    iota = np.tile(np.arange(P, dtype=np.float32), (P, 1))
    return dict(
        FT=FT.astype(f16), whhT=whhT.astype(f16),
        b_rz=np.ascontiguousarray(brz), b_in=np.ascontiguousarray(bin_),
        b_hn=np.ascontiguousarray(bhn),
        iota_f16=iota.astype(f16),
        iota_f32=iota,
        ident_f16=np.eye(P, dtype=np.float32).astype(f16),
    )


def build(meta):
    c = meta["cfg"]
    T, NSEG, SEGROWS, NLOC, STEPS, GT, NGRP = (
        c["TILES"], c["NSEG"], c["SEGROWS"], c["NLOC"], c["STEPS"],
        c["GT"], c["NGRP"])
    GCH = c["GCHUNK"]
    SUBSH = c["SUBSH"]
    TPS = T // NSEG  # tiles per sub-shard
    GPS = NGRP // NSEG  # groups per sub-shard
    ch_b = meta["ch_b"]
    cap_off = meta["cap_off"]
    CAPSUM, CHSUM = meta["CAPSUM"], meta["CHSUM"]

    def bidx(g, s, t_):
        return (g * NSEG + s) * GT + t_

    # per-tile chunk count and per-tile dr col offsets (dr is tile-major)
    chT = np.array([int(sum(ch_b[bidx(i // GT, s, i % GT)]
                            for s in range(NSEG))) for i in range(T)])
    chT_off = np.zeros(T + 1, dtype=np.int64)
    np.cumsum(chT, out=chT_off[1:])
    CTMAX = int(chT.max())
    GMAX = max(int(sum(ch_b[bidx(g, s, t_)] for t_ in range(GT)))
               for g in range(NGRP) for s in range(NSEG))

    nc = bacc.Bacc("TRN2", target_bir_lowering=False, debug=False,
                   num_devices=CORES, num_swdge_queues=4,
                   dynamic_dma_scratch_size=32768)

    idx_d = nc.dram_tensor("idx", [P, CAPSUM // 16], I16, kind="ExternalInput")
    dr_d = nc.dram_tensor("dstrel", [P, CHSUM], F16, kind="ExternalInput")
    xaug_d = nc.dram_tensor("xaug", [T, P, IN_DIM + 1], F16,
                            kind="ExternalInput")
    h0t_d = nc.dram_tensor("h0t", [P, NGRP * 2 * GT * P], F16,
                           kind="ExternalInput")
    h0seg_d = nc.dram_tensor("h0seg", [NSEG * SEGROWS, IN_DIM], F16,
                             kind="ExternalInput")
    batch_d = nc.dram_tensor("batchrel", [P, T], F32, kind="ExternalInput")
    ft_d = nc.dram_tensor("FT", [P, STEPS * 12 * P], F16,
                          kind="ExternalInput")
    whh_d = nc.dram_tensor("whhT", [P, 12 * P], F16, kind="ExternalInput")
    brz_d = nc.dram_tensor("b_rz", [P, 4], F32, kind="ExternalInput")
    bin_d = nc.dram_tensor("b_in", [P, 2], F32, kind="ExternalInput")
    bhn_d = nc.dram_tensor("b_hn", [P, 2], F32, kind="ExternalInput")
    iotah_d = nc.dram_tensor("iota_f16", [P, P], F16, kind="ExternalInput")
    iotaf_d = nc.dram_tensor("iota_f32", [P, P], F32, kind="ExternalInput")
    identh_d = nc.dram_tensor("ident_f16", [P, P], F16, kind="ExternalInput")
    pool_d = nc.dram_tensor("pool", [P, IN_DIM + 1 + HID], F32,
                            kind="ExternalOutput")

    with tile.TileContext(nc) as tc:
        with (
            tc.tile_pool(name="const", bufs=1) as cpool,
            tc.tile_pool(name="stream", bufs=6) as stpool,
            tc.tile_pool(name="gq", bufs=6) as gpool,
            tc.tile_pool(name="Sp", bufs=3) as spool,
            tc.tile_pool(name="work", bufs=3) as wpool,
            tc.tile_pool(name="dram", bufs=1, space="DRAM") as dpool,
        ):
            nc.gpsimd.load_library(library_config.mlp)

            def load_const(name, dram, shape, dtype):
                t_ = cpool.tile(shape, dtype, name=name)
                nc.sync.dma_start(out=t_[:], in_=dram)
                return t_

            whh_sb = load_const("whh", whh_d[:, :], [P, 12 * P], F16)
            ft_sb = load_const("ft", ft_d[:, :], [P, STEPS * 12 * P], F16)
            brz_sb = load_const("brz", brz_d[:, :], [P, 4], F32)
            bin_sb = load_const("bin", bin_d[:, :], [P, 2], F32)
            bhn_sb = load_const("bhn", bhn_d[:, :], [P, 2], F32)
            iotah_sb = load_const("iotah", iotah_d[:, :], [P, P], F16)
            iotaf_sb = load_const("iotaf", iotaf_d[:, :], [P, P], F32)
            identh_sb = load_const("identh", identh_d[:, :], [P, P], F16)
            batch_sb = load_const("batch", batch_d[:, :], [P, T], F32)

            hT = []  # per group [128, (k, tt, n)] = [128, 512]
            for g in range(NGRP):
                t_ = cpool.tile([P, 2 * GT * P], F16, name=f"hT{g}")
                nc.sync.dma_start(
                    out=t_[:], in_=h0t_d[:, g * 512:(g + 1) * 512])
                hT.append(t_)

            swdge_ctr = [0]

            # bounce buffers for steps 1..STEPS-1 tables
            cc_ins = [[dpool.tile([SUBSH, HID], F16, name=f"cc_in{t}_{k}")
                       for k in range(NSEG)] for t in range(STEPS - 1)]
            cc_outs = [[dpool.tile([SEGROWS, HID], F16,
                                   name=f"cc_out{t}_{k}", addr_space="Shared")
                        for k in range(NSEG)] for t in range(STEPS - 1)]

            with (
                tc.tile_pool(name="accps", bufs=2, space="PSUM") as accps,
                tc.tile_pool(name="gateps", bufs=1, space="PSUM") as gateps,
                tc.tile_pool(name="tps", bufs=2, space="PSUM") as tps,
            ):
                def scatter_group(t, g):
                    """Gathers + S-build + node-major scatter for group g.
                    Returns the GT agg PSUM tiles [128 d, ELEM]."""
                    ELEM = IN_DIM if t == 0 else HID
                    i0 = g * GT
                    dr0 = int(chT_off[i0])
                    dr1 = int(chT_off[i0 + GT])
                    dr_sb = stpool.tile([P, GT * CTMAX], F16, tag="dr",
                                        name=f"dr{t}_{g}")
                    nc.sync.dma_start(out=dr_sb[:, 0:dr1 - dr0],
                                      in_=dr_d[:, dr0:dr1])

                    Gs = []
                    for s in range(NSEG):
                        b0 = bidx(g, s, 0)
                        ic0 = int(cap_off[b0] // 16)
                        ic1 = int(cap_off[b0 + GT] // 16)
                        nch = (int(cap_off[b0 + GT]) - int(cap_off[b0])) // P
                        idx_sb = stpool.tile([P, GMAX * 8], I16, tag="idx",
                                             name=f"ix{t}_{g}_{s}")
                        nc.sync.dma_start(out=idx_sb[:, 0:ic1 - ic0],
                                          in_=idx_d[:, ic0:ic1])
                        G = gpool.tile([P, GMAX * HID], F16, tag="g",
                                       name=f"g{t}_{g}_{s}")
                        src_ap = (h0seg_d[s * SEGROWS:(s + 1) * SEGROWS, :]
                                  if t == 0 else cc_outs[t - 1][s][:, :])
                        c0 = 0
                        while c0 < nch:
                            nsub = min(GCH, nch - c0)
                            nc.gpsimd.dma_gather(
                                G[:, c0 * ELEM:(c0 + nsub) * ELEM]
                                .rearrange("p (j d) -> p j d", d=ELEM),
                                src_ap,
                                idx_sb[:, c0 * 8:(c0 + nsub) * 8],
                                nsub * P, nsub * P, ELEM,
                                queue_num=swdge_ctr[0] % 4)
                            swdge_ctr[0] += 1
                            c0 += nsub
                        Gs.append(G)

                    # one S build per tile (dr is tile-major)
                    S_tiles = []
                    for t_ in range(GT):
                        i = i0 + t_
                        cs = int(chT[i])
                        S_sb = spool.tile([P, CTMAX * P], F16, tag="S",
                                          name=f"S{t}_{g}_{t_}")
                        bdr = int(chT_off[i]) - dr0
                        nc.vector.tensor_tensor(
                            out=S_sb[:, :cs * P]
                            .rearrange("p (j d) -> p j d", d=P),
                            in0=dr_sb[:, bdr:bdr + cs]
                            .to_broadcast([P, cs, P]),
                            in1=iotah_sb[:]
                            .rearrange("p (a b) -> p a b", a=1)
                            .to_broadcast([P, cs, P]),
                            op=mybir.AluOpType.is_equal)
                        S_tiles.append(S_sb)

                    agg_grp = accps.tile([P, GT * HID], F32, tag="acc",
                                         name=f"ag{t}_{g}")
                    for t_ in range(GT):
                        i = i0 + t_
                        nchunks = int(chT[i])
                        done = 0
                        scol = 0
                        for s in range(NSEG):
                            b = bidx(g, s, t_)
                            cs = int(ch_b[b])
                            joff = (int(cap_off[b]) -
                                    int(cap_off[bidx(g, s, 0)])) // P
                            for j in range(cs):
                                nc.tensor.matmul(
                                    out=agg_grp[:, t_ * HID:t_ * HID + ELEM],
                                    lhsT=S_tiles[t_][:, (scol + j) * P:
                                                     (scol + j + 1) * P],
                                    rhs=Gs[s][:, (joff + j) * ELEM:
                                              (joff + j + 1) * ELEM],
                                    start=(done == 0),
                                    stop=(done == nchunks - 1),
                                    skip_group_check=True)
                                done += 1
                            scol += cs
                    return agg_grp

                def process_group(t, g, agg_grp):
                    """Transpose agg -> feature-major, GRU for group g, and
                    (t < STEPS-1) write node-major h tiles + launch AG."""
                    ELEM = IN_DIM if t == 0 else HID
                    K = 1 if t == 0 else 2
                    # node-major agg -> SBUF
                    agg_sb = []
                    for t_ in range(GT):
                        a_sb = wpool.tile([P, HID], F16, tag="aggnm",
                                          name=f"an{t}_{g}_{t_}")
                        nc.scalar.copy(out=a_sb[:, 0:ELEM],
                                       in_=agg_grp[:, t_ * HID:
                                                   t_ * HID + ELEM])
                        agg_sb.append(a_sb)
                    # transpose to aggT [128 f, (k, tt, n)]
                    aggT_ps = tps.tile([P, 2 * GT * P], F16, tag="tp",
                                       name=f"aTp{t}_{g}")
                    for k in range(K):
                        for t_ in range(GT):
                            nc.tensor.transpose(
                                out=aggT_ps[:, (k * GT + t_) * P:
                                            (k * GT + t_ + 1) * P],
                                in_=agg_sb[t_][:, k * P:(k + 1) * P],
                                identity=identh_sb[:])
                    aggT = wpool.tile([P, 2 * GT * P], F16, tag="aggT",
                                      name=f"aT{t}_{g}")
                    nc.scalar.copy(out=aggT[:, 0:K * GT * P],
                                   in_=aggT_ps[:, 0:K * GT * P])

                    ftoff = t * 12 * P
                    GN = GT * P  # nodes per group
                    # r,z gates: PSUM [128, (c, tt, n)] in two banks
                    rz_ps = gateps.tile([P, 4 * GN], F32, tag="rz",
                                        name=f"rz{t}_{g}")
                    for cch in range(4):
                        o = rz_ps[:, cch * GN:(cch + 1) * GN]
                        first = True
                        for k in range(K):
                            nc.tensor.matmul(
                                out=o,
                                lhsT=ft_sb[:, ftoff + (k * 6 + cch) * P:
                                           ftoff + (k * 6 + cch + 1) * P],
                                rhs=aggT[:, k * GN:(k + 1) * GN],
                                start=first, stop=False,
                                skip_group_check=True)
                            first = False
                        for k in range(2):
                            nc.tensor.matmul(
                                out=o,
                                lhsT=whh_sb[:, (k * 6 + cch) * P:
                                            (k * 6 + cch + 1) * P],
                                rhs=hT[g][:, k * GN:(k + 1) * GN],
                                start=False, stop=(k == 1),
                                skip_group_check=True)
                    # n gate: i_n and h_n halves
                    inhn_ps = gateps.tile([P, 4 * GN], F32, tag="n",
                                          name=f"n{t}_{g}")
                    for cch in range(2):
                        o = inhn_ps[:, cch * GN:(cch + 1) * GN]
                        first = True
                        for k in range(K):
                            nc.tensor.matmul(
                                out=o,
                                lhsT=ft_sb[:, ftoff + (k * 6 + 4 + cch) * P:
                                           ftoff + (k * 6 + 4 + cch + 1) * P],
                                rhs=aggT[:, k * GN:(k + 1) * GN],
                                start=first, stop=(k == K - 1),
                                skip_group_check=True)
                            first = False
                    for cch in range(2):
                        o = inhn_ps[:, (2 + cch) * GN:(3 + cch) * GN]
                        for k in range(2):
                            nc.tensor.matmul(
                                out=o,
                                lhsT=whh_sb[:, (k * 6 + 4 + cch) * P:
                                            (k * 6 + 4 + cch + 1) * P],
                                rhs=hT[g][:, k * GN:(k + 1) * GN],
                                start=(k == 0), stop=(k == 1),
                                skip_group_check=True)

                    # sigmoid(rz + b) fused on scalar engine
                    rz_sb = wpool.tile([P, 4 * GN], F16, tag="rzsb",
                                       name=f"rzs{t}_{g}")
                    for cch in range(4):
                        nc.scalar.activation(
                            out=rz_sb[:, cch * GN:(cch + 1) * GN],
                            in_=rz_ps[:, cch * GN:(cch + 1) * GN],
                            func=mybir.ActivationFunctionType.Sigmoid,
                            bias=brz_sb[:, cch:cch + 1])
                    # inb = i_n + b_in ; hnb = h_n + b_hn (fused copies)
                    inb = wpool.tile([P, 2 * GN], F16, tag="inb",
                                     name=f"ib{t}_{g}")
                    hnb = wpool.tile([P, 2 * GN], F16, tag="hnb",
                                     name=f"hb{t}_{g}")
                    nc.vector.tensor_tensor(
                        out=inb[:].rearrange("p (c n) -> p c n", n=GN),
                        in0=inhn_ps[:, 0:2 * GN]
                        .rearrange("p (c n) -> p c n", n=GN),
                        in1=bin_sb[:].rearrange("p (c o) -> p c o", o=1)
                        .to_broadcast([P, 2, GN]),
                        op=mybir.AluOpType.add)
                    nc.vector.tensor_tensor(
                        out=hnb[:].rearrange("p (c n) -> p c n", n=GN),
                        in0=inhn_ps[:, 2 * GN:4 * GN]
                        .rearrange("p (c n) -> p c n", n=GN),
                        in1=bhn_sb[:].rearrange("p (c o) -> p c o", o=1)
                        .to_broadcast([P, 2, GN]),
                        op=mybir.AluOpType.add)

                    t1 = wpool.tile([P, 2 * GN], F16, tag="tmp",
                                    name=f"t1{t}_{g}", bufs=6)
                    nc.vector.tensor_mul(out=t1[:], in0=rz_sb[:, 0:2 * GN],
                                         in1=hnb[:])
                    t2 = wpool.tile([P, 2 * GN], F16, tag="tmp",
                                    name=f"t2{t}_{g}", bufs=6)
                    nc.vector.tensor_add(out=t2[:], in0=t1[:], in1=inb[:])
                    n_sb = wpool.tile([P, 2 * GN], F16, tag="tmp",
                                      name=f"ns{t}_{g}", bufs=6)
                    nc.scalar.activation(
                        out=n_sb[:], in_=t2[:],
                        func=mybir.ActivationFunctionType.Tanh)
                    d_sb = wpool.tile([P, 2 * GN], F16, tag="tmp",
                                      name=f"d{t}_{g}", bufs=6)
                    nc.vector.tensor_sub(out=d_sb[:], in0=hT[g][:],
                                         in1=n_sb[:])
                    zd = wpool.tile([P, 2 * GN], F16, tag="tmp",
                                    name=f"zd{t}_{g}", bufs=6)
                    nc.vector.tensor_mul(out=zd[:], in0=rz_sb[:, 2 * GN:],
                                         in1=d_sb[:])
                    nc.vector.tensor_add(out=hT[g][:], in0=zd[:], in1=n_sb[:])

                    if t < STEPS - 1:
                        # node-major h tiles -> bounce buffer
                        for t_ in range(GT):
                            i = g * GT + t_
                            hf_ps = tps.tile([P, 2 * GT * P], F16, tag="tp",
                                             name=f"hf{t}_{g}_{t_}")
                            for k in range(2):
                                nc.tensor.transpose(
                                    out=hf_ps[:, k * P:(k + 1) * P],
                                    in_=hT[g][:, k * GN + t_ * P:
                                              k * GN + (t_ + 1) * P],
                                    identity=identh_sb[:])
                            hn_sb = wpool.tile([P, HID], F16, tag="hnode",
                                               name=f"hns{t}_{g}_{t_}")
                            nc.scalar.copy(out=hn_sb[:], in_=hf_ps[:, 0:HID])
                            ksub = i // TPS
                            ioff = (i % TPS) * P
                            nc.sync.dma_start(
                                out=cc_ins[t][ksub][ioff:ioff + P, :],
                                in_=hn_sb[:])
                        if (g + 1) % GPS == 0:
                            ksub = g // GPS
                            nc.gpsimd.collective_compute(
                                "AllGather", mybir.AluOpType.bypass,
                                replica_groups=[list(range(CORES))],
                                ins=[cc_ins[t][ksub].opt()],
                                outs=[cc_outs[t][ksub].opt()])

                for t in range(STEPS):
                    prev = None
                    for g in range(NGRP):
                        aggs = scatter_group(t, g)
                        if prev is not None:
                            pg, paggs = prev
                            process_group(t, pg, paggs)
                        prev = (g, aggs)
                    pg, paggs = prev
                    process_group(t, pg, paggs)

            # ---- phase C: per-graph pooling partials ----
            with (
                tc.tile_pool(name="poolps", bufs=1, space="PSUM") as ppsC,
                tc.tile_pool(name="hfps", bufs=2, space="PSUM") as hfps,
            ):
                px_ps = ppsC.tile([P, IN_DIM + 1], F32, name="px")
                ph_ps = ppsC.tile([P, HID], F32, name="ph")
                for i in range(T):
                    g, t_ = i // GT, i % GT
                    GN = GT * P
                    hf_ps = hfps.tile([P, HID], F16, tag="hfc", name=f"hf{i}")
                    for k in range(2):
                        nc.tensor.transpose(
                            out=hf_ps[:, k * P:(k + 1) * P],
                            in_=hT[g][:, k * GN + t_ * P:k * GN + (t_ + 1) * P],
                            identity=identh_sb[:])
                    hfin_sb = wpool.tile([P, HID], F16, tag="hfin",
                                         name=f"hfs{i}")
                    nc.scalar.copy(out=hfin_sb[:], in_=hf_ps[:])
                    sg_sb = wpool.tile([P, P], F16, tag="sg", name=f"sg{i}")
                    nc.vector.tensor_scalar(
                        out=sg_sb[:], in0=iotaf_sb[:],
                        scalar1=batch_sb[:, i:i + 1], scalar2=None,
                        op0=mybir.AluOpType.is_equal)
                    xo_sb = stpool.tile([P, IN_DIM + 1], F16, tag="xo",
                                        name=f"xo{i}")
                    nc.sync.dma_start(out=xo_sb[:], in_=xaug_d[i, :, :])
                    nc.tensor.matmul(out=px_ps[:], lhsT=sg_sb[:], rhs=xo_sb[:],
                                     start=(i == 0), stop=(i == T - 1))
                    nc.tensor.matmul(out=ph_ps[:], lhsT=sg_sb[:],
                                     rhs=hfin_sb[:],
                                     start=(i == 0), stop=(i == T - 1))
                px_sb = wpool.tile([P, IN_DIM + 1], F32, name="pxs")
                nc.vector.tensor_copy(out=px_sb[:], in_=px_ps[:])
                ph_sb = wpool.tile([P, HID], F32, name="phs")
                nc.vector.tensor_copy(out=ph_sb[:], in_=ph_ps[:])
                nc.sync.dma_start(out=pool_d[:, 0:IN_DIM + 1], in_=px_sb[:])
                nc.sync.dma_start(out=pool_d[:, IN_DIM + 1:], in_=ph_sb[:])

    nc.compile()
    return nc


_CACHE = {}


def run_device(inputs, cfg, trace=False, tmpdir=None):
    """Preprocess, compile (cached), run on 8 cores; returns per-core pool
    partials [CORES, 128, 385] plus the BassKernelResults."""
    c = _derived(cfg)
    arrays, shared0, meta = preprocess(
        inputs["x"], inputs["edge_index"], inputs["batch"], cfg)
    shared = shared_arrays(
        inputs["ggnn_weight"], inputs["w_ih"], inputs["w_hh"],
        inputs["b_ih"], inputs["b_hh"], c["STEPS"])
    shared.update(shared0)

    key = (tuple(meta["ch_b"].tolist()), c["STEPS"], c["NLOC"], c["GT"],
           c["GCHUNK"])
    if key not in _CACHE:
        _CACHE[key] = build(meta)
    nc = _CACHE[key]

    in_maps = []
    for core in range(CORES):
        m = {k: np.ascontiguousarray(v[core]) for k, v in arrays.items()}
        m.update(shared)
        in_maps.append(m)
    kw = {}
    if trace:
        kw = dict(trace=True, tmpdir=tmpdir)
    res = bass_utils.run_bass_kernel_spmd(
        nc, in_maps, core_ids=list(range(CORES)), **kw)
    pool = np.stack([res.results[cr]["pool"] for cr in range(CORES)])
    return pool, res


def host_epilogue(pool, inputs):
    """Sum per-core partials, mean-pool, and run the classifier MLP."""
    tot = pool.sum(axis=0, dtype=np.float64).astype(np.float32)
    xsum = tot[:, :IN_DIM]
    cnt = tot[:, IN_DIM]
    hsum = tot[:, IN_DIM + 1:]
    feat = np.concatenate([xsum, hsum], axis=1)
    pooled = feat / np.maximum(cnt, 1.0)[:, None]
    w1 = np.asarray(inputs["mlp_w1"], np.float32)
    b1 = np.asarray(inputs["mlp_b1"], np.float32)
    w2 = np.asarray(inputs["mlp_w2"], np.float32)
    b2 = np.asarray(inputs["mlp_b2"], np.float32)
    hdn = np.maximum(pooled @ w1.T + b1, 0.0)
    return (hdn @ w2.T + b2).astype(np.float32)


def kernel(**inputs):
    cfg = _default_cfg()
    pool, _ = run_device(inputs, cfg)
    return host_epilogue(pool, inputs)
